# revision 41
# baseline (speedup 1.0000x reference)
"""Trainium2 Bass kernel for nn_Autoregression (16-state AR whitening log-prob).

Math: reference computes log_prob[b,k,t] = -0.5*(C*log(2pi) + logdet(Sigma_k)
+ es_k(t)^T Sigma_k^{-1} es_k(t)) with es = causal_conv(x, W, b).  Since
Sigma^{-1} = L^{-T} L^{-1} and es is affine in x, fold L^{-1} into the conv:
W2 = L^{-1} W, b2 = L^{-1} b, then mahalanobis = sum_c conv(x; W2, b2)^2.

fp8 DoubleRow version: conv matmuls run in fp8e4 (e4m3) with
perf_mode=DoubleRow, which packs 2 contraction rows per PE cell (virtual
128x256 array).  Contraction of 577 rows (9 taps x 64 cin + bias) per output
chunk is packed as 2 full DR steps of 256 virtual rows (taps 0-7; partition
p = (cin, g), pair slot i covers tap 4s+2g+i) plus one DR leftover step
(tap 8 as channel-pairs on 32 partitions + a ones/bias row).  PSUM chunk is
[128 t, 512 (8 states x 64 ch)] x 2 halves.  ACT squares PSUM -> bf16 SBUF
with the free scale folding in 1/(Sx*Sw*sqrt(2)); DVE does the per-state
segmented reduce with negate ([128,8,64] -> -[128,8]); a small PE transpose
flips [128 t, 16 k] -> [16 k, 128 t] batched 4 chunks per PSUM tile; ACT
adds the per-state constant; DMA out.
"""

import math
import os

import numpy as np
import ml_dtypes

import concourse.bass as bass
import concourse.bacc as bacc_mod
import concourse.mybir as mybir
import concourse.tile as tile
from concourse.bass_utils import run_bass_kernel_spmd
from concourse.tile_rust import add_dep_helper

K = 16          # states
C = 64          # channels
T = 65536       # time
AR = 8          # ar order (kernel size AR+1)
NCORES = 8
TLOC = T // NCORES          # 8192 outputs per core
TC = 128                    # outputs per chunk (matmul M)
WAVE = 16                   # chunks per wave (input tile granularity)
WCOLS = TC * WAVE           # 2048 outputs per wave
NW = TLOC // WCOLS          # waves per core
NH = 2                      # psum halves (states 0-7, 8-15)
NS = 2                      # full DoubleRow contraction steps (taps 0-7)
MTGRP = 4                   # chunks batched per [16, 512] transpose psum

# DoubleRow LDWEIGHTS requires the pair-region byte stride % 16 == 0
XWW = WCOLS + 16            # xq wave-tile region width (max col 2051 used)
XQW = (NW - 1) * WCOLS + XWW  # xq dram region width

FP8 = mybir.dt.float8e4
SQ_DT = mybir.dt.bfloat16   # squares dtype
DR = mybir.MatmulPerfMode.DoubleRow

SX = 16.0                   # x scale into fp8
SW = 64.0                   # weight scale into fp8
ACT_SCALE = 1.0 / (SX * SW * math.sqrt(2.0))

_FP8_NP = ml_dtypes.float8_e4m3
FP8_MAX = 240.0

_CACHE: dict = {}


def _build_program():
    nc = bacc_mod.Bacc()
    f32 = mybir.dt.float32

    # xq rows p=(c,g): 2 regions i: x[c, t0-8 + a + 2g + i] * SX (taps 0-7)
    xq = nc.declare_dram_parameter("xq", [128, 2, XQW], FP8, isOutput=False)
    # xe rows p<32: 2 regions i: x[2p+i, t0 + a] * SX (tap 8);
    # row 32: (ones, zeros); rows 33-63: zeros; rows 64-127: replica of 0-63
    # (leftover matmuls of adjacent chunks run concurrently in PE row-groups
    # {0,1} and {2,3} via tile_position, so both operand copies are needed)
    xe = nc.declare_dram_parameter("xe", [128, 2, TLOC], FP8, isOutput=False)
    # moving operands store DoubleRow pairs interleaved (contiguous byte
    # pairs stream at full rate; split regions force 2 fetches/cycle)
    # wts[p, s, n, i] = SW * W2[k(n), d(n), c(p), 4s + 2g(p) + i]
    wts = nc.declare_dram_parameter("wts", [128, NS, 1024, 2], FP8, isOutput=False)
    # web[p, n, i] = SW * W2[k, d, 2p+i, 8] (p<32); row 32 i=0: SX*SW*b2;
    # rows 64-127: replica of 0-63
    web = nc.declare_dram_parameter("web", [128, 1024, 2], FP8, isOutput=False)
    ident = nc.declare_dram_parameter("ident", [128, 128], mybir.dt.float32r, isOutput=False)
    biasc = nc.declare_dram_parameter("biasc", [K, 1], f32, isOutput=False)
    out = nc.declare_dram_parameter("out", [K, TLOC], f32, isOutput=True)

    with tile.TileContext(nc) as tc:
        with (
            tc.tile_pool(name="singles", bufs=1) as singles,
            # one slot per wave: input DMAs never wait (no slot WAR/WAW)
            tc.tile_pool(name="xpool", bufs=NW) as xpool,
            tc.tile_pool(name="sqpool", bufs=12) as sqpool,
            tc.tile_pool(name="mpool", bufs=10) as mpool,
            tc.tile_pool(name="conv_ps", bufs=6, space="PSUM") as conv_ps,
            tc.tile_pool(name="mt_ps", bufs=1, space="PSUM") as mt_ps,
            tc.tile_pool(name="obs_ps", bufs=1, space="PSUM") as obs_ps,
        ):
            # Matmuls must never be the first PE instruction to observe more
            # than one producer semaphore (1-wait ISA slots; bacc's event-sem
            # legalization costs sequencer time).  pe_observe() emits a tiny
            # 2x2 "reader" matmul whose operands come from a single
            # producer's tile; ordering edges pin readers ahead of the next
            # real matmul.
            scratch = obs_ps.tile([2, 128], f32)
            scratch2 = singles.tile([2, 128], SQ_DT)
            nc.vector.memset(scratch2, 0.0)
            pending = []
            obs_after = [None]

            def pe_observe(col):
                i = nc.tensor.matmul(
                    scratch[0:2, 0:2], col, col, start=True, stop=True
                )
                if obs_after[0] is not None:
                    # not earlier than late in the previous wave, or the PE
                    # FIFO head-of-line blocks on a DMA that hasn't landed
                    add_dep_helper(i.ins, obs_after[0].ins, sync=False)
                pending.append(i)

            def _flush(i):
                while pending:
                    add_dep_helper(i.ins, pending.pop().ins, sync=False)
                return i

            # the tile scheduler reorders PE instructions by priority, which
            # scatters stationary-operand switches; chain every real PE op in
            # emission order so LDWEIGHTS double-buffering can hide switches
            chain = [None]

            def _chain(i):
                if chain[0] is not None:
                    add_dep_helper(i.ins, chain[0].ins, sync=False)
                chain[0] = i
                return i

            def pe_matmul(*args, **kw):
                return _chain(_flush(nc.tensor.matmul(*args, **kw)))

            # dep-free warmup matmuls: keep the PE busy through the initial
            # input DMAs so HAM un-throttles before real work
            for _ in range(50):
                nc.tensor.matmul(
                    scratch[0:2, 0:128],
                    scratch2[0:2, 0:2],
                    scratch2[0:2, 0:128],
                    start=True,
                    stop=True,
                )

            # DMA issue plan: sync HWDGE ring carries the critical path
            # (first xq piece, weights, rest of xq); prefetchables
            # (identity, bias, xe/web, waves 1+) go on the scalar ring.
            w_sb = singles.tile([128, NS, 1024, 2], FP8)
            web_sb = singles.tile([128, 1024, 2], FP8)
            ident_sb = singles.tile([128, 128], mybir.dt.float32r)
            bias_sb = singles.tile([K, 1], f32)
            out_sb = singles.tile([K, TLOC], f32)
            xqs, xes = [], []
            sc_dmas = []
            sc_dmas.append(nc.scalar.dma_start(out=bias_sb, in_=biasc[:, :]))
            for w in range(NW):
                base = w * WCOLS
                xq_w = xpool.tile([128, 2, XWW], FP8, name="xq_w")
                xe_w = xpool.tile([128, 2, WCOLS], FP8, name="xe_w")
                if w == 0:
                    # first piece covers chunks 0-1 (cols < 260); weights are
                    # split per step so chunk-0 s0 waits only on its slice
                    nc.sync.dma_start(out=xq_w[:, :, 0:264], in_=xq[:, :, 0:264])
                    for s in range(NS):
                        nc.sync.dma_start(
                            out=w_sb[:, s, :, :], in_=wts[:, s, :, :]
                        )
                    w_mv = w_sb.rearrange("p s n i -> p s i n")
                    web_mv = web_sb.rearrange("p n i -> p i n")
                    nc.sync.dma_start(
                        out=xq_w[:, :, 264:XWW], in_=xq[:, :, 264:XWW]
                    )
                    # scalar-ring priority order: the first chunk-pair's
                    # leftovers gate the PE chain, so their operands (xe
                    # quarter 0, web rows) come before ident (first needed
                    # by the first transpose batch, much later)
                    for q in range(4):
                        qo = q * (WCOLS // 4)
                        sc_dmas.append(
                            nc.scalar.dma_start(
                                out=xe_w[:, :, qo : qo + WCOLS // 4],
                                in_=xe[:, :, base + qo : base + qo + WCOLS // 4],
                            )
                        )
                        if q == 0:
                            sc_dmas.append(
                                nc.scalar.dma_start(
                                    out=web_sb[0:64], in_=web[0:64, :, :]
                                )
                            )
                            sc_dmas.append(
                                nc.scalar.dma_start(
                                    out=web_sb[64:128], in_=web[64:128, :, :]
                                )
                            )
                            sc_dmas.append(
                                nc.scalar.dma_start(out=ident_sb, in_=ident[:, :])
                            )
                elif w == 1:
                    sc_dmas.append(
                        nc.scalar.dma_start(
                            out=xq_w, in_=xq[:, :, base : base + XWW]
                        )
                    )
                    sc_dmas.append(
                        nc.scalar.dma_start(
                            out=xe_w, in_=xe[:, :, base : base + WCOLS]
                        )
                    )
                xqs.append(xq_w)
                xes.append(xe_w)

            def load_wave_inputs(w):
                # waves 2-3 load lazily (two waves ahead) so the prefetch
                # doesn't flood the DMA fabric while wave 0 computes
                base = w * WCOLS
                nc.scalar.dma_start(out=xqs[w], in_=xq[:, :, base : base + XWW])
                nc.scalar.dma_start(out=xes[w], in_=xe[:, :, base : base + WCOLS])

            # ACT observer for the bias DMA so the first Identity (which
            # also waits on its mt transpose) fits the one-wait ISA slot
            act_scratch = singles.tile([K, 1], f32)
            nc.scalar.copy(act_scratch, bias_sb)

            first_sq = [True]
            m_sbs = []
            pending_tails = []

            def chunk_tail(cglob, psh):
                m_sb = mpool.tile([128, K], mybir.dt.float32r, name="m_sb")
                # two squares (one per PSUM bank) into one sq tile, then a
                # single [128, 16, 64] reduce per chunk (fewer DVE ops)
                sq = sqpool.tile([128, 1024], SQ_DT, name="sq", tag="sq")
                for h in range(NH):
                    sq_i = nc.scalar.activation(
                        sq[:, 512 * h : 512 * h + 512],
                        psh[h],
                        mybir.ActivationFunctionType.Square,
                        scale=ACT_SCALE,
                    )
                    if first_sq[0]:
                        # the Act sequencer must issue every prefetch DMA
                        # before its first square, else a square that
                        # transitively gates one of those DMAs deadlocks
                        while sc_dmas:
                            add_dep_helper(sq_i.ins, sc_dmas.pop().ins, sync=False)
                        first_sq[0] = False
                with nc.allow_low_precision(
                    reason="float32r shares float32 bits; r-mode only "
                    "affects the PE multiply path"
                ):
                    nc.vector.tensor_reduce(
                        out=m_sb[:, 0:K],
                        in_=sq.rearrange("p (g c) -> p g c", g=K),
                        axis=mybir.AxisListType.X,
                        op=mybir.AluOpType.add,
                        negate=True,
                    )
                m_sbs.append(m_sb)
                if cglob % MTGRP == MTGRP - 1:
                    pending_tails.append((cglob - MTGRP + 1, list(m_sbs)))
                    m_sbs.clear()

            def emit_tail():
                # batched transposes (ident stationary loaded once per group),
                # chained into the PE stream a pair late so the DVE reduces
                # they read have landed and the PE never stalls on them
                gbase_c, msbs = pending_tails.pop(0)
                mt = mt_ps.tile([K, MTGRP * TC], mybir.dt.float32r, name="mt")
                for g in range(MTGRP):
                    _chain(
                        _flush(
                            nc.tensor.transpose(
                                mt[:, g * TC : (g + 1) * TC], msbs[g], ident_sb
                            )
                        )
                    )
                gbase = gbase_c * TC
                # out = -m/2 + (-0.5*(Dlog2pi + logdet))  on ACT
                nc.scalar.activation(
                    out_sb[:, gbase : gbase + MTGRP * TC],
                    mt[0:K, :],
                    mybir.ActivationFunctionType.Identity,
                    bias=bias_sb,
                    scale=1.0,
                )
                # the store must be emitted after the ACT write or the tile
                # framework orders it before (WAR) and ships stale columns
                nc.sync.dma_start(
                    out=out[:, gbase : gbase + MTGRP * TC],
                    in_=out_sb[:, gbase : gbase + MTGRP * TC],
                )

            for w in range(NW):
                base = w * WCOLS
                xq_w = xqs[w]
                xe_w = xes[w]
                if w + 2 < NW:
                    load_wave_inputs(w + 2)
                for pc in range(WAVE // 2):
                    cpair = (2 * pc, 2 * pc + 1)
                    psc = {}
                    for cc in cpair:
                        off = cc * TC
                        psc[cc] = [
                            conv_ps.tile(
                                [128, 512], mybir.dt.float32, name=f"ps{h}", tag="ps"
                            )
                            for h in range(NH)
                        ]
                        if cc == 0:
                            if w == 0:
                                pe_observe(ident_sb[:, 0:2])
                            pe_observe(xq_w[0:2, 0, 0:2])
                        for s in range(NS):
                            lhsT = xq_w[:, :, off + 4 * s : off + 4 * s + TC]
                            for h in range(NH):
                                pe_matmul(
                                    psc[cc][h],
                                    lhsT,
                                    w_mv[:, s, :, 512 * h : 512 * h + 512],
                                    start=(s == 0),
                                    stop=False,
                                    perf_mode=DR,
                                )
                    if pc == 0:
                        # lazily: s0-s1 must not stall on the xe/web loads
                        pe_observe(xe_w[0:2, 0, 0:2])
                        pe_observe(web_sb[0:2, 0, 0:2])
                    # leftovers (tap 8 + bias): K=64 row-group tiles; chunk
                    # pair runs concurrently in PE row-groups {0,1} / {2,3}
                    for h in range(NH):
                        for ci, cc in enumerate(cpair):
                            off = cc * TC
                            bp = 64 * ci
                            mm_i = pe_matmul(
                                psc[cc][h],
                                xe_w[bp : bp + 64, :, off : off + TC],
                                web_mv[bp : bp + 64, :, 512 * h : 512 * h + 512],
                                start=False,
                                stop=True,
                                perf_mode=DR,
                                tile_position=(bp, 0),
                            )
                    if pc == WAVE // 2 - 1:
                        obs_after[0] = mm_i
                    for cc in cpair:
                        chunk_tail(w * WAVE + cc, psc[cc])
                    # emit lagged transpose batches; drain fully at the end
                    last = w == NW - 1 and pc == WAVE // 2 - 1
                    while pending_tails and (len(pending_tails) > 1 or last):
                        emit_tail()
    nc.compile()
    return nc


def _prep_host(W, b, Sigma):
    """Fold L^{-1} into conv weights; pack fp8 DoubleRow tiles, constants."""
    W64 = W.astype(np.float64)
    b64 = b.astype(np.float64)
    S64 = Sigma.astype(np.float64)
    L = np.linalg.cholesky(S64)
    Li = np.linalg.inv(L)                       # [K, C, C] lower-triangular inv
    logdet = 2.0 * np.sum(np.log(np.diagonal(L, axis1=1, axis2=2)), axis=1)
    W2 = np.einsum("kdc,kcij->kdij", Li, W64)   # [K, C(d), C(ci), 9]
    b2 = np.einsum("kdc,kc->kd", Li, b64)       # [K, C]

    def q8(v):
        return np.clip(v, -FP8_MAX, FP8_MAX).astype(_FP8_NP)

    # weight column layout: n = 512*(k//8) + 64*(k%8) + d
    W2n = np.transpose(W2, (0, 1, 3, 2)).reshape(K * C, 9, C)  # [(k,d), j, c]
    W2n = W2n.reshape(2, 512, 9, C)                            # [h, n', j, c]

    # wts[p, i, s, n] = SW * W2[k(n), d(n), c(p), 4s + 2g(p) + i]
    wts_np = np.zeros((128, 2, NS, 1024), np.float64)
    for g in range(2):
        for i in range(2):
            for s in range(NS):
                j = 4 * s + 2 * g + i
                # [c, (h, n')] for tap j
                wj = np.transpose(W2n[:, :, j, :], (2, 0, 1)).reshape(C, 1024)
                wts_np[64 * g : 64 * g + 64, i, s, :] = SW * wj
    # web[p, i, n] = SW * W2[k, d, 2p+i, 8] (p<32); row 32 i=0: SX*SW*b2
    web_np = np.zeros((128, 2, 1024), np.float64)
    w8 = np.transpose(W2n[:, :, 8, :], (2, 0, 1)).reshape(C, 1024)  # [c, n]
    web_np[0:32, 0, :] = SW * w8[0::2, :]
    web_np[0:32, 1, :] = SW * w8[1::2, :]
    web_np[32, 0, :] = SX * SW * b2.reshape(2, 8, 64).reshape(1024)
    web_np[64:128] = web_np[0:64]

    const = C * np.log(2.0 * np.pi) + logdet
    bias_np = (-0.5 * const).astype(np.float32).reshape(K, 1)
    # interleave DoubleRow pairs along the innermost byte
    wts_il = np.ascontiguousarray(np.transpose(wts_np, (0, 2, 3, 1)))
    web_il = np.ascontiguousarray(np.transpose(web_np, (0, 2, 1)))
    return q8(wts_il), q8(web_il), bias_np


def _run(x, W, b, Sigma, trace=False):
    x = np.asarray(x, np.float32)
    W = np.asarray(W, np.float32)
    b = np.asarray(b, np.float32)
    Sigma = np.asarray(Sigma, np.float32)
    if "nc" not in _CACHE:
        _CACHE["nc"] = _build_program()
    nc = _CACHE["nc"]
    wts_np, web_np, bias_np = _prep_host(W, b, Sigma)

    # causal left pad (AR) plus right pad so shifted copies stay in bounds
    xp = np.pad(x[0].astype(np.float64), ((0, 0), (AR, 24)))       # [C, T+32]
    xp8 = np.clip(SX * xp, -FP8_MAX, FP8_MAX).astype(_FP8_NP)
    ident_np = np.eye(128, dtype=np.float32)
    in_maps = []
    for ci in range(NCORES):
        t0 = ci * TLOC
        # xq[p, i, a] = xp8[c, t0 + a + 2g + i]
        xq_np = np.zeros((128, 2, XQW), _FP8_NP)
        for g in range(2):
            for i in range(2):
                sh = 2 * g + i
                xq_np[64 * g : 64 * g + 64, i, :] = xp8[:, t0 + sh : t0 + sh + XQW]
        # xe[p, i, a] = xp8[2p+i, t0 + 8 + a] (p<32); row 32 = (1, 0);
        # rows 64-127 replicate 0-63 for the second PE row-group
        xe_np = np.zeros((128, 2, TLOC), _FP8_NP)
        xe_np[0:32, 0, :] = xp8[0::2, t0 + 8 : t0 + 8 + TLOC]
        xe_np[0:32, 1, :] = xp8[1::2, t0 + 8 : t0 + 8 + TLOC]
        xe_np[32, 0, :] = _FP8_NP(1.0)
        xe_np[64:128] = xe_np[0:64]
        in_maps.append(
            {
                "xq": xq_np,
                "xe": xe_np,
                "wts": wts_np,
                "web": web_np,
                "ident": ident_np,
                "biasc": bias_np,
            }
        )
    res = run_bass_kernel_spmd(
        nc, in_maps, core_ids=list(range(NCORES)), trace=trace
    )
    outs = [res.results[i]["out"] for i in range(NCORES)]
    full = np.concatenate(outs, axis=1)[None]   # [1, K, T]
    return full.astype(np.float32), res


def kernel(x, W, b, Sigma):
    out, _ = _run(x, W, b, Sigma, trace=bool(int(os.environ.get("BASS_TRACE", "0"))))
    return out


# revision 43
# speedup vs baseline: 1.0376x; 1.0376x over previous
"""Trainium2 Bass kernel for nn_Autoregression (16-state AR whitening log-prob).

Math: reference computes log_prob[b,k,t] = -0.5*(C*log(2pi) + logdet(Sigma_k)
+ es_k(t)^T Sigma_k^{-1} es_k(t)) with es = causal_conv(x, W, b).  Since
Sigma^{-1} = L^{-T} L^{-1} and es is affine in x, fold L^{-1} into the conv:
W2 = L^{-1} W, b2 = L^{-1} b, then mahalanobis = sum_c conv(x; W2, b2)^2.

fp8 DoubleRow version: conv matmuls run in fp8e4 (e4m3) with
perf_mode=DoubleRow, which packs 2 contraction rows per PE cell (virtual
128x256 array).  Contraction of 577 rows (9 taps x 64 cin + bias) per output
chunk is packed as 2 full DR steps of 256 virtual rows (taps 0-7; partition
p = (cin, g), pair slot i covers tap 4s+2g+i) plus one DR leftover step
(tap 8 as channel-pairs on 32 partitions + a ones/bias row).  PSUM chunk is
[128 t, 512 (8 states x 64 ch)] x 2 halves.  ACT squares PSUM -> bf16 SBUF
with the free scale folding in 1/(Sx*Sw*sqrt(2)); DVE does the per-state
segmented reduce with negate ([128,8,64] -> -[128,8]); a small PE transpose
flips [128 t, 16 k] -> [16 k, 128 t] batched 4 chunks per PSUM tile; ACT
adds the per-state constant; DMA out.
"""

import math
import os

import numpy as np
import ml_dtypes

import concourse.bass as bass
import concourse.bacc as bacc_mod
import concourse.mybir as mybir
import concourse.tile as tile
from concourse.bass_utils import run_bass_kernel_spmd
from concourse.tile_rust import add_dep_helper

K = 16          # states
C = 64          # channels
T = 65536       # time
AR = 8          # ar order (kernel size AR+1)
NCORES = 8
TLOC = T // NCORES          # 8192 outputs per core
TC = 128                    # outputs per chunk (matmul M)
WAVE = 16                   # chunks per wave (input tile granularity)
WCOLS = TC * WAVE           # 2048 outputs per wave
NW = TLOC // WCOLS          # waves per core
NH = 2                      # psum halves (states 0-7, 8-15)
NS = 2                      # full DoubleRow contraction steps (taps 0-7)
MTGRP = 4                   # chunks batched per [16, 512] transpose psum

# DoubleRow LDWEIGHTS requires the pair-region byte stride % 16 == 0
XWW = WCOLS + 16            # xq wave-tile region width (max col 2051 used)
XQW = (NW - 1) * WCOLS + XWW  # xq dram region width

FP8 = mybir.dt.float8e4
SQ_DT = mybir.dt.bfloat16   # squares dtype
DR = mybir.MatmulPerfMode.DoubleRow

SX = 16.0                   # x scale into fp8
SW = 64.0                   # weight scale into fp8
ACT_SCALE = 1.0 / (SX * SW * math.sqrt(2.0))

_FP8_NP = ml_dtypes.float8_e4m3
FP8_MAX = 240.0

_CACHE: dict = {}


def _build_program():
    nc = bacc_mod.Bacc()
    f32 = mybir.dt.float32

    # xq rows p=(c,g): 2 regions i: x[c, t0-8 + a + 2g + i] * SX (taps 0-7)
    xq = nc.declare_dram_parameter("xq", [128, 2, XQW], FP8, isOutput=False)
    # xe rows p<32: 2 regions i: x[2p+i, t0 + a] * SX (tap 8);
    # row 32: (ones, zeros); rows 33-63: zeros; rows 64-127: replica of 0-63
    # (leftover matmuls of adjacent chunks run concurrently in PE row-groups
    # {0,1} and {2,3} via tile_position, so both operand copies are needed)
    xe = nc.declare_dram_parameter("xe", [128, 2, TLOC], FP8, isOutput=False)
    # moving operands store DoubleRow pairs interleaved (contiguous byte
    # pairs stream at full rate; split regions force 2 fetches/cycle)
    # wts[p, s, n, i] = SW * W2[k(n), d(n), c(p), 4s + 2g(p) + i]
    wts = nc.declare_dram_parameter("wts", [128, NS, 1024, 2], FP8, isOutput=False)
    # web[p, n, i] = SW * W2[k, d, 2p+i, 8] (p<32); row 32 i=0: SX*SW*b2;
    # rows 64-127: replica of 0-63
    web = nc.declare_dram_parameter("web", [128, 1024, 2], FP8, isOutput=False)
    ident = nc.declare_dram_parameter("ident", [128, 128], mybir.dt.float32r, isOutput=False)
    biasc = nc.declare_dram_parameter("biasc", [K, 1], f32, isOutput=False)
    out = nc.declare_dram_parameter("out", [K, TLOC], f32, isOutput=True)

    with tile.TileContext(nc) as tc:
        with (
            tc.tile_pool(name="singles", bufs=1) as singles,
            # one slot per wave: input DMAs never wait (no slot WAR/WAW)
            tc.tile_pool(name="xpool", bufs=NW) as xpool,
            tc.tile_pool(name="sqpool", bufs=12) as sqpool,
            tc.tile_pool(name="mpool", bufs=10) as mpool,
            tc.tile_pool(name="conv_ps", bufs=6, space="PSUM") as conv_ps,
            tc.tile_pool(name="mt_ps", bufs=1, space="PSUM") as mt_ps,
            tc.tile_pool(name="obs_ps", bufs=1, space="PSUM") as obs_ps,
        ):
            # Matmuls must never be the first PE instruction to observe more
            # than one producer semaphore (1-wait ISA slots; bacc's event-sem
            # legalization costs sequencer time).  pe_observe() emits a tiny
            # 2x2 "reader" matmul whose operands come from a single
            # producer's tile; ordering edges pin readers ahead of the next
            # real matmul.
            scratch = obs_ps.tile([2, 128], f32)
            scratch2 = singles.tile([2, 128], SQ_DT)
            nc.vector.memset(scratch2, 0.0)
            pending = []
            obs_after = [None]

            def pe_observe(col):
                i = nc.tensor.matmul(
                    scratch[0:2, 0:2], col, col, start=True, stop=True
                )
                if obs_after[0] is not None:
                    # not earlier than late in the previous wave, or the PE
                    # FIFO head-of-line blocks on a DMA that hasn't landed
                    add_dep_helper(i.ins, obs_after[0].ins, sync=False)
                pending.append(i)

            def _flush(i):
                while pending:
                    add_dep_helper(i.ins, pending.pop().ins, sync=False)
                return i

            # the tile scheduler reorders PE instructions by priority, which
            # scatters stationary-operand switches; chain every real PE op in
            # emission order so LDWEIGHTS double-buffering can hide switches
            chain = [None]

            def _chain(i):
                if chain[0] is not None:
                    add_dep_helper(i.ins, chain[0].ins, sync=False)
                chain[0] = i
                return i

            def pe_matmul(*args, **kw):
                return _chain(_flush(nc.tensor.matmul(*args, **kw)))

            # dep-free warmup matmuls: keep the PE busy through the initial
            # input DMAs so HAM un-throttles before real work
            for _ in range(50):
                nc.tensor.matmul(
                    scratch[0:2, 0:128],
                    scratch2[0:2, 0:2],
                    scratch2[0:2, 0:128],
                    start=True,
                    stop=True,
                )

            # DMA issue plan: sync HWDGE ring carries the critical path
            # (first xq piece, weights, rest of xq); prefetchables
            # (identity, bias, xe/web, waves 1+) go on the scalar ring.
            w_sb = singles.tile([128, NS, 1024, 2], FP8)
            web_sb = singles.tile([128, 1024, 2], FP8)
            ident_sb = singles.tile([128, 128], mybir.dt.float32r)
            bias_sb = singles.tile([K, 1], f32)
            out_sb = singles.tile([K, TLOC], f32)
            xqs, xes = [], []
            sc_dmas = []
            sc_dmas.append(nc.scalar.dma_start(out=bias_sb, in_=biasc[:, :]))
            for w in range(NW):
                base = w * WCOLS
                xq_w = xpool.tile([128, 2, XWW], FP8, name="xq_w")
                xe_w = xpool.tile([128, 2, WCOLS], FP8, name="xe_w")
                if w == 0:
                    # first piece covers chunks 0-1 (cols < 260); weights are
                    # split per step so chunk-0 s0 waits only on its slice
                    nc.sync.dma_start(out=xq_w[:, :, 0:264], in_=xq[:, :, 0:264])
                    for s in range(NS):
                        nc.sync.dma_start(
                            out=w_sb[:, s, :, :], in_=wts[:, s, :, :]
                        )
                    w_mv = w_sb.rearrange("p s n i -> p s i n")
                    web_mv = web_sb.rearrange("p n i -> p i n")
                    nc.sync.dma_start(
                        out=xq_w[:, :, 264:XWW], in_=xq[:, :, 264:XWW]
                    )
                    # scalar-ring priority order: the first chunk-pair's
                    # leftovers gate the PE chain, so their operands (xe
                    # quarter 0, web rows) come before ident (first needed
                    # by the first transpose batch, much later)
                    for q in range(4):
                        qo = q * (WCOLS // 4)
                        sc_dmas.append(
                            nc.scalar.dma_start(
                                out=xe_w[:, :, qo : qo + WCOLS // 4],
                                in_=xe[:, :, base + qo : base + qo + WCOLS // 4],
                            )
                        )
                        if q == 0:
                            sc_dmas.append(
                                nc.scalar.dma_start(
                                    out=web_sb[0:64], in_=web[0:64, :, :]
                                )
                            )
                            sc_dmas.append(
                                nc.scalar.dma_start(
                                    out=web_sb[64:128], in_=web[64:128, :, :]
                                )
                            )
                            sc_dmas.append(
                                nc.scalar.dma_start(out=ident_sb, in_=ident[:, :])
                            )
                elif w == 1:
                    sc_dmas.append(
                        nc.scalar.dma_start(
                            out=xq_w, in_=xq[:, :, base : base + XWW]
                        )
                    )
                    sc_dmas.append(
                        nc.scalar.dma_start(
                            out=xe_w, in_=xe[:, :, base : base + WCOLS]
                        )
                    )
                xqs.append(xq_w)
                xes.append(xe_w)

            def load_wave_inputs(w):
                # waves 2-3 load lazily (two waves ahead) so the prefetch
                # doesn't flood the DMA fabric while wave 0 computes
                base = w * WCOLS
                nc.scalar.dma_start(out=xqs[w], in_=xq[:, :, base : base + XWW])
                nc.scalar.dma_start(out=xes[w], in_=xe[:, :, base : base + WCOLS])

            # ACT observer for the bias DMA so the first Identity (which
            # also waits on its mt transpose) fits the one-wait ISA slot
            act_scratch = singles.tile([K, 1], f32)
            nc.scalar.copy(act_scratch, bias_sb)

            first_sq = [True]
            m_sbs = []
            pending_tails = []

            def chunk_tail(cglob, psh):
                m_sb = mpool.tile([128, K], mybir.dt.float32r, name="m_sb")
                # two squares (one per PSUM bank) into one sq tile, then a
                # single [128, 16, 64] reduce per chunk (fewer DVE ops)
                sq = sqpool.tile([128, 1024], SQ_DT, name="sq", tag="sq")
                for h in range(NH):
                    sq_i = nc.scalar.activation(
                        sq[:, 512 * h : 512 * h + 512],
                        psh[h],
                        mybir.ActivationFunctionType.Square,
                        scale=ACT_SCALE,
                    )
                    if first_sq[0]:
                        # the Act sequencer must issue every prefetch DMA
                        # before its first square, else a square that
                        # transitively gates one of those DMAs deadlocks
                        while sc_dmas:
                            add_dep_helper(sq_i.ins, sc_dmas.pop().ins, sync=False)
                        first_sq[0] = False
                with nc.allow_low_precision(
                    reason="float32r shares float32 bits; r-mode only "
                    "affects the PE multiply path"
                ):
                    nc.vector.tensor_reduce(
                        out=m_sb[:, 0:K],
                        in_=sq.rearrange("p (g c) -> p g c", g=K),
                        axis=mybir.AxisListType.X,
                        op=mybir.AluOpType.add,
                        negate=True,
                    )
                m_sbs.append(m_sb)
                if cglob % MTGRP == MTGRP - 1:
                    pending_tails.append((cglob - MTGRP + 1, list(m_sbs)))
                    m_sbs.clear()

            first_tail = [True]

            def emit_tail():
                # batched transposes (ident stationary loaded once per group),
                # chained into the PE stream a pair late so the DVE reduces
                # they read have landed and the PE never stalls on them
                if first_tail[0]:
                    pe_observe(ident_sb[:, 0:2])
                    first_tail[0] = False
                gbase_c, msbs = pending_tails.pop(0)
                mt = mt_ps.tile([K, MTGRP * TC], mybir.dt.float32r, name="mt")
                for g in range(MTGRP):
                    _chain(
                        _flush(
                            nc.tensor.transpose(
                                mt[:, g * TC : (g + 1) * TC], msbs[g], ident_sb
                            )
                        )
                    )
                gbase = gbase_c * TC
                # out = -m/2 + (-0.5*(Dlog2pi + logdet))  on ACT
                nc.scalar.activation(
                    out_sb[:, gbase : gbase + MTGRP * TC],
                    mt[0:K, :],
                    mybir.ActivationFunctionType.Identity,
                    bias=bias_sb,
                    scale=1.0,
                )
                # the store must be emitted after the ACT write or the tile
                # framework orders it before (WAR) and ships stale columns
                nc.sync.dma_start(
                    out=out[:, gbase : gbase + MTGRP * TC],
                    in_=out_sb[:, gbase : gbase + MTGRP * TC],
                )

            for w in range(NW):
                base = w * WCOLS
                xq_w = xqs[w]
                xe_w = xes[w]
                if w + 2 < NW:
                    load_wave_inputs(w + 2)
                for pc in range(WAVE // 2):
                    cpair = (2 * pc, 2 * pc + 1)
                    psc = {}
                    for cc in cpair:
                        off = cc * TC
                        psc[cc] = [
                            conv_ps.tile(
                                [128, 512], mybir.dt.float32, name=f"ps{h}", tag="ps"
                            )
                            for h in range(NH)
                        ]
                        if cc == 0:
                            pe_observe(xq_w[0:2, 0, 0:2])
                        for s in range(NS):
                            lhsT = xq_w[:, :, off + 4 * s : off + 4 * s + TC]
                            for h in range(NH):
                                pe_matmul(
                                    psc[cc][h],
                                    lhsT,
                                    w_mv[:, s, :, 512 * h : 512 * h + 512],
                                    start=(s == 0),
                                    stop=False,
                                    perf_mode=DR,
                                )
                    if pc == 0:
                        # lazily: s0-s1 must not stall on the xe/web loads
                        pe_observe(xe_w[0:2, 0, 0:2])
                        pe_observe(web_sb[0:2, 0, 0:2])
                    # leftovers (tap 8 + bias): K=64 row-group tiles; chunk
                    # pair runs concurrently in PE row-groups {0,1} / {2,3}
                    for h in range(NH):
                        for ci, cc in enumerate(cpair):
                            off = cc * TC
                            bp = 64 * ci
                            mm_i = pe_matmul(
                                psc[cc][h],
                                xe_w[bp : bp + 64, :, off : off + TC],
                                web_mv[bp : bp + 64, :, 512 * h : 512 * h + 512],
                                start=False,
                                stop=True,
                                perf_mode=DR,
                                tile_position=(bp, 0),
                            )
                    if pc == WAVE // 2 - 1:
                        obs_after[0] = mm_i
                    for cc in cpair:
                        chunk_tail(w * WAVE + cc, psc[cc])
                    # emit lagged transpose batches; drain fully at the end
                    last = w == NW - 1 and pc == WAVE // 2 - 1
                    while pending_tails and (len(pending_tails) > 1 or last):
                        emit_tail()
    nc.compile()
    return nc


def _prep_host(W, b, Sigma):
    """Fold L^{-1} into conv weights; pack fp8 DoubleRow tiles, constants."""
    W64 = W.astype(np.float64)
    b64 = b.astype(np.float64)
    S64 = Sigma.astype(np.float64)
    L = np.linalg.cholesky(S64)
    Li = np.linalg.inv(L)                       # [K, C, C] lower-triangular inv
    logdet = 2.0 * np.sum(np.log(np.diagonal(L, axis1=1, axis2=2)), axis=1)
    W2 = np.einsum("kdc,kcij->kdij", Li, W64)   # [K, C(d), C(ci), 9]
    b2 = np.einsum("kdc,kc->kd", Li, b64)       # [K, C]

    def q8(v):
        return np.clip(v, -FP8_MAX, FP8_MAX).astype(_FP8_NP)

    # weight column layout: n = 512*(k//8) + 64*(k%8) + d
    W2n = np.transpose(W2, (0, 1, 3, 2)).reshape(K * C, 9, C)  # [(k,d), j, c]
    W2n = W2n.reshape(2, 512, 9, C)                            # [h, n', j, c]

    # wts[p, i, s, n] = SW * W2[k(n), d(n), c(p), 4s + 2g(p) + i]
    wts_np = np.zeros((128, 2, NS, 1024), np.float64)
    for g in range(2):
        for i in range(2):
            for s in range(NS):
                j = 4 * s + 2 * g + i
                # [c, (h, n')] for tap j
                wj = np.transpose(W2n[:, :, j, :], (2, 0, 1)).reshape(C, 1024)
                wts_np[64 * g : 64 * g + 64, i, s, :] = SW * wj
    # web[p, i, n] = SW * W2[k, d, 2p+i, 8] (p<32); row 32 i=0: SX*SW*b2
    web_np = np.zeros((128, 2, 1024), np.float64)
    w8 = np.transpose(W2n[:, :, 8, :], (2, 0, 1)).reshape(C, 1024)  # [c, n]
    web_np[0:32, 0, :] = SW * w8[0::2, :]
    web_np[0:32, 1, :] = SW * w8[1::2, :]
    web_np[32, 0, :] = SX * SW * b2.reshape(2, 8, 64).reshape(1024)
    web_np[64:128] = web_np[0:64]

    const = C * np.log(2.0 * np.pi) + logdet
    bias_np = (-0.5 * const).astype(np.float32).reshape(K, 1)
    # interleave DoubleRow pairs along the innermost byte
    wts_il = np.ascontiguousarray(np.transpose(wts_np, (0, 2, 3, 1)))
    web_il = np.ascontiguousarray(np.transpose(web_np, (0, 2, 1)))
    return q8(wts_il), q8(web_il), bias_np


def _run(x, W, b, Sigma, trace=False):
    x = np.asarray(x, np.float32)
    W = np.asarray(W, np.float32)
    b = np.asarray(b, np.float32)
    Sigma = np.asarray(Sigma, np.float32)
    if "nc" not in _CACHE:
        _CACHE["nc"] = _build_program()
    nc = _CACHE["nc"]
    wts_np, web_np, bias_np = _prep_host(W, b, Sigma)

    # causal left pad (AR) plus right pad so shifted copies stay in bounds
    xp = np.pad(x[0].astype(np.float64), ((0, 0), (AR, 24)))       # [C, T+32]
    xp8 = np.clip(SX * xp, -FP8_MAX, FP8_MAX).astype(_FP8_NP)
    ident_np = np.eye(128, dtype=np.float32)
    in_maps = []
    for ci in range(NCORES):
        t0 = ci * TLOC
        # xq[p, i, a] = xp8[c, t0 + a + 2g + i]
        xq_np = np.zeros((128, 2, XQW), _FP8_NP)
        for g in range(2):
            for i in range(2):
                sh = 2 * g + i
                xq_np[64 * g : 64 * g + 64, i, :] = xp8[:, t0 + sh : t0 + sh + XQW]
        # xe[p, i, a] = xp8[2p+i, t0 + 8 + a] (p<32); row 32 = (1, 0);
        # rows 64-127 replicate 0-63 for the second PE row-group
        xe_np = np.zeros((128, 2, TLOC), _FP8_NP)
        xe_np[0:32, 0, :] = xp8[0::2, t0 + 8 : t0 + 8 + TLOC]
        xe_np[0:32, 1, :] = xp8[1::2, t0 + 8 : t0 + 8 + TLOC]
        xe_np[32, 0, :] = _FP8_NP(1.0)
        xe_np[64:128] = xe_np[0:64]
        in_maps.append(
            {
                "xq": xq_np,
                "xe": xe_np,
                "wts": wts_np,
                "web": web_np,
                "ident": ident_np,
                "biasc": bias_np,
            }
        )
    res = run_bass_kernel_spmd(
        nc, in_maps, core_ids=list(range(NCORES)), trace=trace
    )
    outs = [res.results[i]["out"] for i in range(NCORES)]
    full = np.concatenate(outs, axis=1)[None]   # [1, K, T]
    return full.astype(np.float32), res


def kernel(x, W, b, Sigma):
    out, _ = _run(x, W, b, Sigma, trace=bool(int(os.environ.get("BASS_TRACE", "0"))))
    return out


# revision 48
# speedup vs baseline: 1.2351x; 1.1903x over previous
"""Trainium2 Bass kernel for nn_Autoregression (16-state AR whitening log-prob).

Math: reference computes log_prob[b,k,t] = -0.5*(C*log(2pi) + logdet(Sigma_k)
+ es_k(t)^T Sigma_k^{-1} es_k(t)) with es = causal_conv(x, W, b).  Since
Sigma^{-1} = L^{-T} L^{-1} and es is affine in x, fold L^{-1} into the conv:
W2 = L^{-1} W, b2 = L^{-1} b, then mahalanobis = sum_c conv(x; W2, b2)^2.

fp8 DoubleRow version: conv matmuls run in fp8e4 (e4m3) with
perf_mode=DoubleRow, which packs 2 contraction rows per PE cell (virtual
128x256 array).  Contraction of 577 rows (9 taps x 64 cin + bias) per output
chunk is packed as 2 full DR steps of 256 virtual rows (taps 0-7; partition
p = (cin, g), pair slot i covers tap 4s+2g+i) plus one DR leftover step
(tap 8 as channel-pairs on 32 partitions + a ones/bias row).  PSUM chunk is
[128 t, 512 (8 states x 64 ch)] x 2 halves.  ACT squares PSUM -> bf16 SBUF
with the free scale folding in 1/(Sx*Sw*sqrt(2)); DVE does the per-state
segmented reduce with negate ([128,8,64] -> -[128,8]); a small PE transpose
flips [128 t, 16 k] -> [16 k, 128 t] batched 4 chunks per PSUM tile; ACT
adds the per-state constant; DMA out.
"""

import math
import os

import numpy as np
import ml_dtypes

import concourse.bass as bass
import concourse.bacc as bacc_mod
import concourse.mybir as mybir
import concourse.tile as tile
from concourse.bass_utils import run_bass_kernel_spmd
from concourse.tile_rust import add_dep_helper

K = 16          # states
C = 64          # channels
T = 65536       # time
AR = 8          # ar order (kernel size AR+1)
NCORES = 8
TLOC = T // NCORES          # 8192 outputs per core
TC = 128                    # outputs per chunk (matmul M)
WAVE = 16                   # chunks per wave (input tile granularity)
WCOLS = TC * WAVE           # 2048 outputs per wave
NW = TLOC // WCOLS          # waves per core
NH = 2                      # psum halves (states 0-7, 8-15)
NS = 2                      # full DoubleRow contraction steps (taps 0-7)
MTGRP = 4                   # chunks batched per [16, 512] transpose psum

# DoubleRow LDWEIGHTS requires the pair-region byte stride % 16 == 0
XWW = WCOLS + 16            # xq wave-tile region width (max col 2051 used)
XQW = (NW - 1) * WCOLS + XWW  # xq dram region width

FP8 = mybir.dt.float8e4
SQ_DT = mybir.dt.bfloat16   # squares dtype
DR = mybir.MatmulPerfMode.DoubleRow

SX = 16.0                   # x scale into fp8
SW = 64.0                   # weight scale into fp8
ACT_SCALE = 1.0 / (SX * SW * math.sqrt(2.0))

_FP8_NP = ml_dtypes.float8_e4m3
FP8_MAX = 240.0

_CACHE: dict = {}


def _build_program():
    nc = bacc_mod.Bacc()
    f32 = mybir.dt.float32

    # xq rows p=(c,g): 2 regions i: x[c, t0-8 + a + 2g + i] * SX (taps 0-7)
    xq = nc.declare_dram_parameter("xq", [128, 2, XQW], FP8, isOutput=False)
    # xe rows p<32: 2 regions i: x[2p+i, t0 + a] * SX (tap 8);
    # row 32: (ones, zeros); rows 33-63: zeros; rows 64-127: replica of 0-63
    # (leftover matmuls of adjacent chunks run concurrently in PE row-groups
    # {0,1} and {2,3} via tile_position, so both operand copies are needed)
    xe = nc.declare_dram_parameter("xe", [128, 2, TLOC], FP8, isOutput=False)
    # moving operands store DoubleRow pairs interleaved (contiguous byte
    # pairs stream at full rate; split regions force 2 fetches/cycle)
    # wts[p, s, n, i] = SW * W2[k(n), d(n), c(p), 4s + 2g(p) + i]
    wts = nc.declare_dram_parameter("wts", [128, NS, 1024, 2], FP8, isOutput=False)
    # web[p, n, i] = SW * W2[k, d, 2p+i, 8] (p<32); row 32 i=0: SX*SW*b2;
    # rows 64-127: replica of 0-63
    web = nc.declare_dram_parameter("web", [128, 1024, 2], FP8, isOutput=False)
    ident = nc.declare_dram_parameter("ident", [128, 128], mybir.dt.float32r, isOutput=False)
    biasc = nc.declare_dram_parameter("biasc", [K, 1], f32, isOutput=False)
    out = nc.declare_dram_parameter("out", [K, TLOC], f32, isOutput=True)

    with tile.TileContext(nc) as tc:
        with (
            tc.tile_pool(name="singles", bufs=1) as singles,
            # one slot per wave: input DMAs never wait (no slot WAR/WAW)
            tc.tile_pool(name="xpool", bufs=NW) as xpool,
            tc.tile_pool(name="sqpool", bufs=12) as sqpool,
            tc.tile_pool(name="mpool", bufs=10) as mpool,
            tc.tile_pool(name="conv_ps", bufs=6, space="PSUM") as conv_ps,
            tc.tile_pool(name="mt_ps", bufs=1, space="PSUM") as mt_ps,
            tc.tile_pool(name="obs_ps", bufs=1, space="PSUM") as obs_ps,
        ):
            # Matmuls must never be the first PE instruction to observe more
            # than one producer semaphore (1-wait ISA slots; bacc's event-sem
            # legalization costs sequencer time).  pe_observe() emits a tiny
            # 2x2 "reader" matmul whose operands come from a single
            # producer's tile; ordering edges pin readers ahead of the next
            # real matmul.
            scratch = obs_ps.tile([2, 128], f32)
            scratch2 = singles.tile([2, 128], SQ_DT)
            nc.vector.memset(scratch2, 0.0)
            pending = []
            obs_after = [None]

            def pe_observe(col):
                i = nc.tensor.matmul(
                    scratch[0:2, 0:2], col, col, start=True, stop=True
                )
                if obs_after[0] is not None:
                    # not earlier than late in the previous wave, or the PE
                    # FIFO head-of-line blocks on a DMA that hasn't landed
                    add_dep_helper(i.ins, obs_after[0].ins, sync=False)
                pending.append(i)

            def _flush(i):
                while pending:
                    add_dep_helper(i.ins, pending.pop().ins, sync=False)
                return i

            # the tile scheduler reorders PE instructions by priority, which
            # scatters stationary-operand switches; chain every real PE op in
            # emission order so LDWEIGHTS double-buffering can hide switches
            chain = [None]

            def _chain(i):
                if chain[0] is not None:
                    add_dep_helper(i.ins, chain[0].ins, sync=False)
                chain[0] = i
                return i

            def pe_matmul(*args, **kw):
                return _chain(_flush(nc.tensor.matmul(*args, **kw)))

            # dep-free warmup matmuls: keep the PE busy through the initial
            # input DMAs so HAM un-throttles before real work
            for _ in range(35):
                nc.tensor.matmul(
                    scratch[0:2, 0:128],
                    scratch2[0:2, 0:2],
                    scratch2[0:2, 0:128],
                    start=True,
                    stop=True,
                )

            # DMA issue plan: sync HWDGE ring carries the critical path
            # (first xq piece, weights, rest of xq); prefetchables
            # (identity, bias, xe/web, waves 1+) go on the scalar ring.
            w_sb = singles.tile([128, NS, 1024, 2], FP8)
            web_sb = singles.tile([128, 1024, 2], FP8)
            ident_sb = singles.tile([128, 128], mybir.dt.float32r)
            bias_sb = singles.tile([K, 1], f32)
            out_sb = singles.tile([K, TLOC], f32)
            xqs, xes = [], []
            sc_dmas = []
            sc_dmas.append(nc.scalar.dma_start(out=ident_sb, in_=ident[:, :]))
            sc_dmas.append(nc.scalar.dma_start(out=bias_sb, in_=biasc[:, :]))
            for w in range(NW):
                base = w * WCOLS
                xq_w = xpool.tile([128, 2, XWW], FP8, name="xq_w")
                xe_w = xpool.tile([128, 2, WCOLS], FP8, name="xe_w")
                if w == 0:
                    # first piece covers chunks 0-1 (cols < 260); weights are
                    # split per step so chunk-0 s0 waits only on its slice
                    nc.sync.dma_start(out=xq_w[:, :, 0:264], in_=xq[:, :, 0:264])
                    for s in range(NS):
                        nc.sync.dma_start(
                            out=w_sb[:, s, :, :], in_=wts[:, s, :, :]
                        )
                    w_mv = w_sb.rearrange("p s n i -> p s i n")
                    web_mv = web_sb.rearrange("p n i -> p i n")
                    nc.sync.dma_start(
                        out=xq_w[:, :, 264:XWW], in_=xq[:, :, 264:XWW]
                    )
                    for q in range(4):
                        qo = q * (WCOLS // 4)
                        sc_dmas.append(
                            nc.scalar.dma_start(
                                out=xe_w[:, :, qo : qo + WCOLS // 4],
                                in_=xe[:, :, base + qo : base + qo + WCOLS // 4],
                            )
                        )
                        if q == 0:
                            sc_dmas.append(
                                nc.scalar.dma_start(out=web_sb, in_=web[:, :, :])
                            )
                elif w == 1:
                    sc_dmas.append(
                        nc.scalar.dma_start(
                            out=xq_w, in_=xq[:, :, base : base + XWW]
                        )
                    )
                    sc_dmas.append(
                        nc.scalar.dma_start(
                            out=xe_w, in_=xe[:, :, base : base + WCOLS]
                        )
                    )
                xqs.append(xq_w)
                xes.append(xe_w)

            def load_wave_inputs(w):
                # waves 2-3 load lazily (two waves ahead) so the prefetch
                # doesn't flood the DMA fabric while wave 0 computes
                base = w * WCOLS
                nc.scalar.dma_start(out=xqs[w], in_=xq[:, :, base : base + XWW])
                nc.scalar.dma_start(out=xes[w], in_=xe[:, :, base : base + WCOLS])

            # ACT observer for the bias DMA so the first Identity (which
            # also waits on its mt transpose) fits the one-wait ISA slot
            act_scratch = singles.tile([K, 1], f32)
            nc.scalar.copy(act_scratch, bias_sb)

            first_sq = [True]
            m_sbs = []
            pending_tails = []

            def chunk_tail(cglob, psh):
                m_sb = mpool.tile([128, K], mybir.dt.float32r, name="m_sb")
                # two squares (one per PSUM bank) into one sq tile, then a
                # single [128, 16, 64] reduce per chunk (fewer DVE ops)
                sq = sqpool.tile([128, 1024], SQ_DT, name="sq", tag="sq")
                for h in range(NH):
                    sq_i = nc.scalar.activation(
                        sq[:, 512 * h : 512 * h + 512],
                        psh[h],
                        mybir.ActivationFunctionType.Square,
                        scale=ACT_SCALE,
                    )
                    if first_sq[0]:
                        # the Act sequencer must issue every prefetch DMA
                        # before its first square, else a square that
                        # transitively gates one of those DMAs deadlocks
                        while sc_dmas:
                            add_dep_helper(sq_i.ins, sc_dmas.pop().ins, sync=False)
                        first_sq[0] = False
                with nc.allow_low_precision(
                    reason="float32r shares float32 bits; r-mode only "
                    "affects the PE multiply path"
                ):
                    nc.vector.tensor_reduce(
                        out=m_sb[:, 0:K],
                        in_=sq.rearrange("p (g c) -> p g c", g=K),
                        axis=mybir.AxisListType.X,
                        op=mybir.AluOpType.add,
                        negate=True,
                    )
                m_sbs.append(m_sb)
                if cglob % MTGRP == MTGRP - 1:
                    pending_tails.append((cglob - MTGRP + 1, list(m_sbs)))
                    m_sbs.clear()

            def emit_tail():
                # batched transposes (ident stationary loaded once per group),
                # chained into the PE stream a pair late so the DVE reduces
                # they read have landed and the PE never stalls on them
                gbase_c, msbs = pending_tails.pop(0)
                mt = mt_ps.tile([K, MTGRP * TC], mybir.dt.float32r, name="mt")
                for g in range(MTGRP):
                    _chain(
                        _flush(
                            nc.tensor.transpose(
                                mt[:, g * TC : (g + 1) * TC], msbs[g], ident_sb
                            )
                        )
                    )
                gbase = gbase_c * TC
                # out = -m/2 + (-0.5*(Dlog2pi + logdet))  on ACT
                nc.scalar.activation(
                    out_sb[:, gbase : gbase + MTGRP * TC],
                    mt[0:K, :],
                    mybir.ActivationFunctionType.Identity,
                    bias=bias_sb,
                    scale=1.0,
                )
                # the store must be emitted after the ACT write or the tile
                # framework orders it before (WAR) and ships stale columns
                nc.sync.dma_start(
                    out=out[:, gbase : gbase + MTGRP * TC],
                    in_=out_sb[:, gbase : gbase + MTGRP * TC],
                )

            for w in range(NW):
                base = w * WCOLS
                xq_w = xqs[w]
                xe_w = xes[w]
                if w + 2 < NW:
                    load_wave_inputs(w + 2)
                for pc in range(WAVE // 2):
                    cpair = (2 * pc, 2 * pc + 1)
                    psc = {}
                    for cc in cpair:
                        off = cc * TC
                        psc[cc] = [
                            conv_ps.tile(
                                [128, 512], mybir.dt.float32, name=f"ps{h}", tag="ps"
                            )
                            for h in range(NH)
                        ]
                        if cc == 0:
                            if w == 0:
                                pe_observe(ident_sb[:, 0:2])
                            pe_observe(xq_w[0:2, 0, 0:2])
                        for s in range(NS):
                            lhsT = xq_w[:, :, off + 4 * s : off + 4 * s + TC]
                            for h in range(NH):
                                pe_matmul(
                                    psc[cc][h],
                                    lhsT,
                                    w_mv[:, s, :, 512 * h : 512 * h + 512],
                                    start=(s == 0),
                                    stop=False,
                                    perf_mode=DR,
                                )
                    if pc == 0:
                        # lazily: s0-s1 must not stall on the xe/web loads
                        pe_observe(xe_w[0:2, 0, 0:2])
                        pe_observe(web_sb[0:2, 0, 0:2])
                    # leftovers (tap 8 + bias): K=64 row-group tiles; chunk
                    # pair runs concurrently in PE row-groups {0,1} / {2,3}
                    for h in range(NH):
                        for ci, cc in enumerate(cpair):
                            off = cc * TC
                            bp = 64 * ci
                            mm_i = pe_matmul(
                                psc[cc][h],
                                xe_w[bp : bp + 64, :, off : off + TC],
                                web_mv[bp : bp + 64, :, 512 * h : 512 * h + 512],
                                start=False,
                                stop=True,
                                perf_mode=DR,
                                tile_position=(bp, 0),
                            )
                    if pc == WAVE // 2 - 1:
                        obs_after[0] = mm_i
                    for cc in cpair:
                        chunk_tail(w * WAVE + cc, psc[cc])
                    # emit lagged transpose batches; drain fully at the end
                    last = w == NW - 1 and pc == WAVE // 2 - 1
                    while pending_tails and (len(pending_tails) > 1 or last):
                        emit_tail()
    nc.compile()
    return nc


def _prep_host(W, b, Sigma):
    """Fold L^{-1} into conv weights; pack fp8 DoubleRow tiles, constants."""
    W64 = W.astype(np.float64)
    b64 = b.astype(np.float64)
    S64 = Sigma.astype(np.float64)
    L = np.linalg.cholesky(S64)
    Li = np.linalg.inv(L)                       # [K, C, C] lower-triangular inv
    logdet = 2.0 * np.sum(np.log(np.diagonal(L, axis1=1, axis2=2)), axis=1)
    W2 = np.einsum("kdc,kcij->kdij", Li, W64)   # [K, C(d), C(ci), 9]
    b2 = np.einsum("kdc,kc->kd", Li, b64)       # [K, C]

    def q8(v):
        return np.clip(v, -FP8_MAX, FP8_MAX).astype(_FP8_NP)

    # weight column layout: n = 512*(k//8) + 64*(k%8) + d
    W2n = np.transpose(W2, (0, 1, 3, 2)).reshape(K * C, 9, C)  # [(k,d), j, c]
    W2n = W2n.reshape(2, 512, 9, C)                            # [h, n', j, c]

    # wts[p, i, s, n] = SW * W2[k(n), d(n), c(p), 4s + 2g(p) + i]
    wts_np = np.zeros((128, 2, NS, 1024), np.float64)
    for g in range(2):
        for i in range(2):
            for s in range(NS):
                j = 4 * s + 2 * g + i
                # [c, (h, n')] for tap j
                wj = np.transpose(W2n[:, :, j, :], (2, 0, 1)).reshape(C, 1024)
                wts_np[64 * g : 64 * g + 64, i, s, :] = SW * wj
    # web[p, i, n] = SW * W2[k, d, 2p+i, 8] (p<32); row 32 i=0: SX*SW*b2
    web_np = np.zeros((128, 2, 1024), np.float64)
    w8 = np.transpose(W2n[:, :, 8, :], (2, 0, 1)).reshape(C, 1024)  # [c, n]
    web_np[0:32, 0, :] = SW * w8[0::2, :]
    web_np[0:32, 1, :] = SW * w8[1::2, :]
    web_np[32, 0, :] = SX * SW * b2.reshape(2, 8, 64).reshape(1024)
    web_np[64:128] = web_np[0:64]

    const = C * np.log(2.0 * np.pi) + logdet
    bias_np = (-0.5 * const).astype(np.float32).reshape(K, 1)
    # interleave DoubleRow pairs along the innermost byte
    wts_il = np.ascontiguousarray(np.transpose(wts_np, (0, 2, 3, 1)))
    web_il = np.ascontiguousarray(np.transpose(web_np, (0, 2, 1)))
    return q8(wts_il), q8(web_il), bias_np


def _run(x, W, b, Sigma, trace=False):
    x = np.asarray(x, np.float32)
    W = np.asarray(W, np.float32)
    b = np.asarray(b, np.float32)
    Sigma = np.asarray(Sigma, np.float32)
    if "nc" not in _CACHE:
        _CACHE["nc"] = _build_program()
    nc = _CACHE["nc"]
    wts_np, web_np, bias_np = _prep_host(W, b, Sigma)

    # causal left pad (AR) plus right pad so shifted copies stay in bounds
    xp = np.pad(x[0].astype(np.float64), ((0, 0), (AR, 24)))       # [C, T+32]
    xp8 = np.clip(SX * xp, -FP8_MAX, FP8_MAX).astype(_FP8_NP)
    ident_np = np.eye(128, dtype=np.float32)
    in_maps = []
    for ci in range(NCORES):
        t0 = ci * TLOC
        # xq[p, i, a] = xp8[c, t0 + a + 2g + i]
        xq_np = np.zeros((128, 2, XQW), _FP8_NP)
        for g in range(2):
            for i in range(2):
                sh = 2 * g + i
                xq_np[64 * g : 64 * g + 64, i, :] = xp8[:, t0 + sh : t0 + sh + XQW]
        # xe[p, i, a] = xp8[2p+i, t0 + 8 + a] (p<32); row 32 = (1, 0);
        # rows 64-127 replicate 0-63 for the second PE row-group
        xe_np = np.zeros((128, 2, TLOC), _FP8_NP)
        xe_np[0:32, 0, :] = xp8[0::2, t0 + 8 : t0 + 8 + TLOC]
        xe_np[0:32, 1, :] = xp8[1::2, t0 + 8 : t0 + 8 + TLOC]
        xe_np[32, 0, :] = _FP8_NP(1.0)
        xe_np[64:128] = xe_np[0:64]
        in_maps.append(
            {
                "xq": xq_np,
                "xe": xe_np,
                "wts": wts_np,
                "web": web_np,
                "ident": ident_np,
                "biasc": bias_np,
            }
        )
    res = run_bass_kernel_spmd(
        nc, in_maps, core_ids=list(range(NCORES)), trace=trace
    )
    outs = [res.results[i]["out"] for i in range(NCORES)]
    full = np.concatenate(outs, axis=1)[None]   # [1, K, T]
    return full.astype(np.float32), res


def kernel(x, W, b, Sigma):
    out, _ = _run(x, W, b, Sigma, trace=bool(int(os.environ.get("BASS_TRACE", "0"))))
    return out
